# revision 20
# baseline (speedup 1.0000x reference)
"""Trainium2 Bass kernel for nn_DepthMemoryCache.

Reference computation (D=8, B=4, S=4096, C=1024, G=64):
    u     = einsum('bsc,gc->bsg', x[-1], W_u)
    keys  = einsum('dbc,gc->dbg', x.mean(2), W_u)
    gates = softmax(einsum('bsg,dbg->bsd', u, keys), axis=-1)
    out   = einsum('dbsc,bsd->bsc', x, gates)

Strategy: shard the sequence axis over 8 cores (core i gets
x[:, :, i*512:(i+1)*512, :]). Per core the kernel is one continuous DMA
stream — both phases are DMA-bound and every compute engine stays well
under the read bandwidth:

  Phase A (batch-major): for each b, stream the 8 depth slabs (d=7
  first). Depths 5..7 are cast (on ACT, the cheapest engine for pure
  copies) to fp16 into a 12MB resident SBUF buffer, so phase B never
  re-reads them. Each slab is 4->1 j-reduced by a small DVE add tree
  into one fp16 plane, so the per-(d,b) column sum needs only TWO N=512
  PE matmuls with a one-hot indicator stationary into a [D, C] PSUM
  region. uT = W_u @ x7T blocks (PE transposes + matmuls off the
  resident fp16 x7) are interleaved one per slab.

  When batch b's slabs finish, its [G, D] partial keys are fixed up
  (transpose + matmuls, ~2us) and AllReduced on its own: 4 tiny 2KB
  collectives, each hidden under the remaining phase-A streaming (a
  warm-up AllReduce absorbs comm setup). GpSimd carries ONLY the cc_in
  bounces + collective_compute ops (a collective parks its queue until
  the fabric completes, so nothing else may ride it); the cc_out
  bounces ride the scalar queue at the start of each phase-B batch,
  after their collective is long done.

  Phase B: streamed reads are issued back-to-back behind phase A's on
  the sync queue. Per 128-row block: one small matmul for logits,
  softmax via ACT exp with accum_out. The depth-weighted sum runs on
  the OTHERWISE-IDLE tensor engine: diag(g_d) stationaries (ident_h
  scaled by the per-partition gate column, one tiny [128,128] DVE op
  each) ride 16 accumulating N=512 fp16 matmuls per block into a
  [P, C] PSUM tile — every DVE/ACT engine is element-rate-bound at
  ~1.1-1.5us per [P, C] pass, so an 8-deep elementwise chain can never
  keep up with DMA, but PE does the whole combine in ~7us/block with
  exact fp32 accumulation. Streamed tiles are pre-cast fp32->fp16
  (split ACT/DVE); the PSUM result is copied out once and DMA'd on the
  scalar HWDGE ring.

HBM traffic per core: 64 (A) + 40 (B reads, 5/8 depths) + 8 (write)
= 112MB. fp16 x/gate quantization costs ~5e-4 relative on a 2e-2
budget.
"""
import sys

sys.path.insert(0, "/opt/trn_rl_repo")

from contextlib import ExitStack

import numpy as np
from concourse import bacc, bass, mybir, tile, masks
from concourse import bass_utils

F32 = mybir.dt.float32
F16 = mybir.dt.float16

D, B, S, C, G = 8, 4, 4096, 1024, 64
N_CORES = 8
P = 128                 # partition count / block rows
NKC = C // P            # 8 column chunks of 128
NRES = 3                # depths D-NRES..D-1 stay resident in SBUF as fp16
RES0 = D - NRES


def build_body(tc, x, w, y, s_sh):
    """Emit the kernel IR. x:[D,B,s_sh,C], w:[G,C], y:[B,s_sh,C] dram APs."""
    nc = tc.nc
    nj = s_sh // P      # 128-row blocks per (d, b)
    mul, add = mybir.AluOpType.mult, mybir.AluOpType.add
    es = ExitStack()

    singles = es.enter_context(tc.tile_pool(name="singles", bufs=1))
    ident = singles.tile([P, P], F32)
    masks.make_identity(nc, ident[:])
    ident_h = singles.tile([P, P], F16)
    masks.make_identity(nc, ident_h[:])
    # indicator stationaries: ind[:, d, m] = (m == d) / S — column-sums a
    # j-reduced fp16 plane into psum row d with one N=512 matmul per c-half.
    ind_h = singles.tile([P, D, D], F16)
    nc.vector.memset(ind_h[:], 0.0)
    for d in range(D):
        nc.vector.memset(ind_h[:, d, d:d + 1], 1.0 / (N_CORES * s_sh))
    w_sb = singles.tile([G, C], F32)
    nc.sync.dma_start(w_sb[:], w[:])
    # resident fp16 depths: xres[:, r] holds depth RES0+r; r=NRES-1 is d=D-1
    xres = singles.tile([P, NRES, B, nj, C], F16)
    gates_sb = singles.tile([P, B, nj, D], F32)
    meanT_sb = singles.tile([P, NKC * D], F32)
    wT_sb = singles.tile([P, NKC, G], F32)
    wT_h = singles.tile([P, NKC, G], F16)
    # per-b tiles: a shared tile would give phase-B readers a whole-tile RAW
    # hazard on the LAST writer (b=3's bounce / u-block), stalling b=0's gates
    keysT_sb = [singles.tile([G, D], F32, name=f"keysT{b}") for b in range(B)]
    uT_sb = [singles.tile([G, nj, P], F32, name=f"uT{b}") for b in range(B)]

    bstream = es.enter_context(tc.tile_pool(name="bstream", bufs=6))

    dram = es.enter_context(tc.tile_pool(name="dram", bufs=1, space="DRAM"))
    # tiny warm-up AllReduce: absorbs collective-comm setup under phase A
    ccw_in = dram.tile([1, 16], F32)
    ccw_out = dram.tile([1, 16], F32)
    cc_in = [dram.tile([G, D], F32, name=f"cc_in{b}") for b in range(B)]
    cc_out = [dram.tile([G, D], F32, name=f"cc_out{b}") for b in range(B)]
    warm_sb = singles.tile([1, 16], F32)
    nc.vector.memset(warm_sb[:], 0.0)
    nc.gpsimd.dma_start(ccw_in[:], warm_sb[:])
    nc.gpsimd.collective_compute(
        "AllReduce", add, replica_groups=[list(range(N_CORES))],
        ins=[ccw_in.opt()], outs=[ccw_out.opt()],
    )

    # ---------------- Phase A: batch-major streaming + per-b collective ----
    with tc.tile_pool(name="psumA", bufs=1, space="PSUM") as psA, \
         tc.tile_pool(name="psumT", bufs=1, space="PSUM") as psT, \
         tc.tile_pool(name="psumXA", bufs=3, space="PSUM") as psXA, \
         tc.tile_pool(name="psumU", bufs=2, space="PSUM") as psU, \
         tc.tile_pool(name="xtA", bufs=4) as xtA, \
         tc.tile_pool(name="stream", bufs=3) as stream, \
         tc.tile_pool(name="jtree", bufs=6) as jtree, \
         tc.tile_pool(name="sumsp", bufs=2) as sumsp, \
         tc.tile_pool(name="ksump", bufs=2) as ksump:

        # one-time W_u transpose: wT[c, g] chunks (fp32 + fp16 copies)
        for k in range(NKC):
            tr = psT.tile([P, NKC * D], F32, tag="fix")
            nc.tensor.transpose(
                tr[:, :G], w_sb[:, k * P:(k + 1) * P], ident[:G, :G])
            nc.vector.tensor_copy(wT_sb[:, k, :], tr[:, :G])
            nc.scalar.copy(wT_h[:, k, :], tr[:, :G])

        sums_ps = psA.tile([D, C], F32)

        def sum_plane(plane_h, d, first, last):
            # psum rows m != d get +0; one start/stop per 512-col bank per b
            for h in range(2):
                nc.tensor.matmul(
                    sums_ps[:, h * 512:(h + 1) * 512],
                    ind_h[:, d, :],
                    plane_h[:, h * 512:(h + 1) * 512],
                    start=first, stop=last,
                )

        def u_block(b, j):
            # uT[g, s-block] = sum_k (wT_k).T @ x7T_k on PE (reads resident
            # x7 fp16, so this can run any time after the d=D-1 cast)
            u_ps = psU.tile([G, P], F32, tag="u")
            for k in range(NKC):
                xt_ps = psXA.tile([P, P], F16, tag="xt_ps")
                nc.tensor.transpose(
                    xt_ps[:], xres[:, NRES - 1, b, j, k * P:(k + 1) * P],
                    ident_h[:])
                xt_sb = xtA.tile([P, P], F16, tag="xt_sb")
                if k % 2 == 0:
                    nc.scalar.copy(xt_sb[:], xt_ps[:])
                else:
                    nc.vector.tensor_copy(xt_sb[:], xt_ps[:])
                nc.tensor.matmul(
                    u_ps[:], wT_h[:, k, :], xt_sb[:],
                    start=(k == 0), stop=(k == NKC - 1))
            nc.vector.tensor_copy(uT_sb[b][:, j, :], u_ps[:])

        # depth order: d=7 first (fills resident x7 for u_blocks); the other
        # resident depths are spread mid-batch so their ACT cast bursts
        # don't pile up at the batch boundary
        dorder = [7, 0, 5, 1, 2, 6, 3, 4]
        assert sorted(dorder) == list(range(D))
        ublocks = [(b, j) for b in range(B) for j in range(nj)]
        ub_i = 0
        for b in range(B):
            for di, d in enumerate(dorder):
                slab = stream.tile([P, nj, C], F32, tag="slab")
                nc.sync.dma_start(
                    slab[:], x[d, b].rearrange("(j p) c -> p j c", p=P))
                t01 = jtree.tile([P, C], F16, tag="jt")
                t23 = jtree.tile([P, C], F16, tag="jt")
                tfin = jtree.tile([P, C], F16, tag="jt")
                if d >= RES0:
                    # resident: fp16 casts into xres on ACT, tree reads fp16
                    r = d - RES0
                    for j in range(nj):
                        nc.scalar.copy(xres[:, r, b, j, :], slab[:, j, :])
                    nc.vector.tensor_tensor(
                        t01[:], xres[:, r, b, 0, :], xres[:, r, b, 1, :], add)
                    nc.vector.tensor_tensor(
                        t23[:], xres[:, r, b, 2, :], xres[:, r, b, 3, :], add)
                else:
                    # transient: fp32 adds with fp16 outputs, no extra cast
                    nc.vector.tensor_tensor(
                        t01[:], slab[:, 0, :], slab[:, 1, :], add)
                    nc.vector.tensor_tensor(
                        t23[:], slab[:, 2, :], slab[:, 3, :], add)
                nc.vector.tensor_tensor(tfin[:], t01[:], t23[:], add)
                sum_plane(tfin[:], d, first=(di == 0), last=(di == D - 1))
                # interleave one uT block every other slab (its x7 input is
                # cast at di=0 of its own batch, so never dispatch ahead)
                if di % 2 == 1 and ub_i < len(ublocks) \
                        and ublocks[ub_i][0] <= b:
                    u_block(*ublocks[ub_i])
                    ub_i += 1

            # ---- per-b fixup: meanT transpose + partial keysT + AllReduce --
            sums_sb = sumsp.tile([D, C], F32, tag="sums")
            nc.vector.tensor_copy(sums_sb[:], sums_ps[:])
            mt_ps = psT.tile([P, NKC * D], F32, tag="fix")
            for k in range(NKC):
                nc.tensor.matmul(
                    mt_ps[:, k * D:(k + 1) * D],
                    sums_sb[:, k * P:(k + 1) * P], ident[:D, :D],
                    is_transpose=True, start=(k == 0), stop=(k == NKC - 1))
            nc.vector.tensor_copy(meanT_sb[:], mt_ps[:])
            keys_ps = psT.tile([P, NKC * D], F32, tag="fix")
            for k in range(NKC):
                nc.tensor.matmul(
                    keys_ps[:G, :D],
                    wT_sb[:, k, :],
                    meanT_sb[:, k * D:(k + 1) * D],
                    start=(k == 0), stop=(k == NKC - 1),
                )
            ksum_sb = ksump.tile([G, D], F32, tag="ksum")
            nc.vector.tensor_copy(ksum_sb[:], keys_ps[:G, :D])
            # gpsimd carries ONLY the collective chain: a collective_compute
            # parks its queue until the fabric completes, so no streaming
            # work may ride behind it
            nc.gpsimd.dma_start(cc_in[b][:], ksum_sb[:])
            nc.gpsimd.collective_compute(
                "AllReduce", add,
                replica_groups=[list(range(N_CORES))],
                ins=[cc_in[b].opt()], outs=[cc_out[b].opt()],
            )

    # ---------------- Phase B: gates + depth-weighted sum -------------------
    # The whole phase is emitted under a far-future wait_until fence: the
    # Tile scheduler's sim thinks collectives are fast and would otherwise
    # hoist collective-dependent phase-B ops (cc_out bounce, logits matmul)
    # into the middle of phase-A engine queues — parking ACT/PE on a 25-200us
    # fabric wait and freezing the stream pipeline behind them.
    es_b = ExitStack()
    es_b.enter_context(tc.tile_wait_until(1.0))
    with tc.tile_pool(name="psumL", bufs=2, space="PSUM") as psL, \
         tc.tile_pool(name="psumO", bufs=2, space="PSUM") as psO, \
         tc.tile_pool(name="acc16", bufs=3) as acc16p, \
         tc.tile_pool(name="accf", bufs=3) as accfp, \
         tc.tile_pool(name="bcast", bufs=8) as bcastp, \
         tc.tile_pool(name="small", bufs=8) as small:
        def emit_gates(b):
            # cc_out bounce on the scalar queue: collective b completed under
            # phase-A streaming (b=3's under phase B's b=2 streaming)
            nc.scalar.dma_start(keysT_sb[b][:], cc_out[b][:])
            for j in range(nj):
                lg_ps = psL.tile([P, D], F32, tag="lg")
                nc.tensor.matmul(lg_ps[:], uT_sb[b][:, j, :], keysT_sb[b][:])
                e_sb = small.tile([P, D], F32, tag="e")
                z_sb = small.tile([P, 1], F32, tag="z")
                rz_sb = small.tile([P, 1], F32, tag="rz")
                nc.scalar.activation(
                    e_sb[:], lg_ps[:], mybir.ActivationFunctionType.Exp,
                    accum_out=z_sb[:])
                nc.vector.reciprocal(rz_sb[:], z_sb[:])
                nc.scalar.mul(gates_sb[:, b, j, :], e_sb[:], rz_sb[:])

        # staggered fences freeze the intended order: with one fence time
        # the scheduler sim sees all 4 bounces "ready" at once and groups
        # them ahead of b=0's gates, parking the ACT queue on b=3's
        # collective. Gates are emitted ONE batch ahead of their combine
        # section so each batch boundary finds its gates precomputed.
        emit_gates(0)
        for b in range(B):
            tc.tile_set_cur_wait(1.0 + 0.5 * (b + 1))
            if b + 1 < B:
                emit_gates(b + 1)
            for j in range(nj):
                # streamed depths: ACT fuses the gate scale into the
                # fp32->fp16 downcast, then PE accumulates them with a plain
                # identity stationary into fp32 PSUM (DVE/ACT are
                # element-rate-bound at ~1.2us per [P, C] pass, so an 8-deep
                # elementwise chain can't keep up with DMA — PE + a short
                # 3-deep DVE chain for the resident depths can).
                out_ps = psO.tile([P, C], F32, tag="out")
                for d in range(RES0):
                    t = bstream.tile([P, C], F32, tag="bslab")
                    nc.sync.dma_start(t[:], x[d, b, j * P:(j + 1) * P, :])
                    th = bcastp.tile([P, C], F16, tag="bc")
                    nc.scalar.activation(
                        th[:], t[:], mybir.ActivationFunctionType.Copy,
                        scale=gates_sb[:, b, j, d:d + 1])
                    for h in range(2):
                        nc.tensor.matmul(
                            out_ps[:, h * 512:(h + 1) * 512],
                            ident_h[:], th[:, h * 512:(h + 1) * 512],
                            start=(d == 0), stop=(d == RES0 - 1))
                # resident depths: short fp16 scalar_tensor_tensor chain
                acc16 = acc16p.tile([P, C], F16, tag="a16")
                nc.vector.tensor_scalar_mul(
                    acc16[:], xres[:, 0, b, j, :], gates_sb[:, b, j, RES0:RES0 + 1])
                for r in range(1, NRES):
                    d = RES0 + r
                    nc.vector.scalar_tensor_tensor(
                        out=acc16[:], in0=xres[:, r, b, j, :],
                        scalar=gates_sb[:, b, j, d:d + 1],
                        in1=acc16[:], op0=mul, op1=add)
                # final fp32 combine drains the PSUM group in the same op
                accf = accfp.tile([P, C], F32, tag="af")
                nc.vector.tensor_tensor(accf[:], out_ps[:], acc16[:], add)
                # y writes via the ACT HWDGE ring: sync keeps reading,
                # gpsimd keeps its collective queue clear
                nc.scalar.dma_start(y[b, j * P:(j + 1) * P, :], accf[:])

    es_b.close()
    es.close()


def build_nc(s_sh):
    nc = bacc.Bacc("TRN2", target_bir_lowering=False, debug=False,
                   num_devices=N_CORES)
    x_ap = nc.dram_tensor("x", [D, B, s_sh, C], F32, kind="ExternalInput").ap()
    w_ap = nc.dram_tensor("w", [G, C], F32, kind="ExternalInput").ap()
    y_ap = nc.dram_tensor("y", [B, s_sh, C], F32, kind="ExternalOutput").ap()
    with tile.TileContext(nc) as tc:
        build_body(tc, x_ap, w_ap, y_ap, s_sh)
    nc.compile()
    return nc


_NC_CACHE = {}


def _get_nc(s_sh):
    if s_sh not in _NC_CACHE:
        _NC_CACHE[s_sh] = build_nc(s_sh)
    return _NC_CACHE[s_sh]


def run(cached_states, W_u, trace=False, trace_cores=None):
    s_sh = S // N_CORES
    nc = _get_nc(s_sh)
    xs = np.asarray(cached_states, dtype=np.float32)
    ws = np.ascontiguousarray(np.asarray(W_u, dtype=np.float32))
    in_maps = []
    for i in range(N_CORES):
        sh = np.ascontiguousarray(xs[:, :, i * s_sh:(i + 1) * s_sh, :])
        in_maps.append({"x": sh, "w": ws})
    res = bass_utils.run_bass_kernel_spmd(
        nc, in_maps, core_ids=list(range(N_CORES)), trace=trace,
        trace_cores=trace_cores)
    out = np.empty((B, S, C), np.float32)
    for i in range(N_CORES):
        out[:, i * s_sh:(i + 1) * s_sh, :] = res.results[i]["y"]
    return out, res


def kernel(cached_states, W_u):
    out, _ = run(cached_states, W_u)
    return out


# revision 25
# speedup vs baseline: 1.0752x; 1.0752x over previous
"""Trainium2 Bass kernel for nn_DepthMemoryCache.

Reference computation (D=8, B=4, S=4096, C=1024, G=64):
    u     = einsum('bsc,gc->bsg', x[-1], W_u)
    keys  = einsum('dbc,gc->dbg', x.mean(2), W_u)
    gates = softmax(einsum('bsg,dbg->bsd', u, keys), axis=-1)
    out   = einsum('dbsc,bsd->bsc', x, gates)

Strategy: shard the sequence axis over 8 cores (core i gets
x[:, :, i*512:(i+1)*512, :]). Per core the kernel is one continuous DMA
stream — both phases are DMA-bound and every compute engine stays well
under the read bandwidth:

  Phase A (batch-major): for each b, stream the 8 depth slabs (d=7
  first). Depths 5..7 are cast (on ACT, the cheapest engine for pure
  copies) to fp16 into a 12MB resident SBUF buffer, so phase B never
  re-reads them. Each slab is 4->1 j-reduced by a small DVE add tree
  into one fp16 plane, so the per-(d,b) column sum needs only TWO N=512
  PE matmuls with a one-hot indicator stationary into a [D, C] PSUM
  region. uT = W_u @ x7T blocks (PE transposes + matmuls off the
  resident fp16 x7) are interleaved one per slab.

  When batch b's slabs finish, its [G, D] partial keys are fixed up
  (transpose + matmuls, ~2us) and AllReduced on its own: 4 tiny 2KB
  collectives, each hidden under the remaining phase-A streaming (a
  warm-up AllReduce absorbs comm setup). GpSimd carries ONLY the cc_in
  bounces + collective_compute ops (a collective parks its queue until
  the fabric completes, so nothing else may ride it); the cc_out
  bounces ride the scalar queue at the start of each phase-B batch,
  after their collective is long done.

  Phase B: streamed reads are issued back-to-back behind phase A's on
  the sync queue. Per 128-row block: one small matmul for logits,
  softmax via ACT exp with accum_out. The depth-weighted sum runs on
  the OTHERWISE-IDLE tensor engine: diag(g_d) stationaries (ident_h
  scaled by the per-partition gate column, one tiny [128,128] DVE op
  each) ride 16 accumulating N=512 fp16 matmuls per block into a
  [P, C] PSUM tile — every DVE/ACT engine is element-rate-bound at
  ~1.1-1.5us per [P, C] pass, so an 8-deep elementwise chain can never
  keep up with DMA, but PE does the whole combine in ~7us/block with
  exact fp32 accumulation. Streamed tiles are pre-cast fp32->fp16
  (split ACT/DVE); the PSUM result is copied out once and DMA'd on the
  scalar HWDGE ring.

HBM traffic per core: 64 (A) + 40 (B reads, 5/8 depths) + 8 (write)
= 112MB. fp16 x/gate quantization costs ~5e-4 relative on a 2e-2
budget.
"""
import sys

sys.path.insert(0, "/opt/trn_rl_repo")

from contextlib import ExitStack

import numpy as np
from concourse import bacc, bass, mybir, tile, masks
from concourse import bass_utils

F32 = mybir.dt.float32
F16 = mybir.dt.float16

D, B, S, C, G = 8, 4, 4096, 1024, 64
N_CORES = 8
P = 128                 # partition count / block rows
NKC = C // P            # 8 column chunks of 128
NRES = 3                # depths D-NRES..D-1 stay resident in SBUF as fp16
RES0 = D - NRES


def build_body(tc, x, w, y, s_sh):
    """Emit the kernel IR. x:[D,B,s_sh,C], w:[G,C], y:[B,s_sh,C] dram APs."""
    nc = tc.nc
    nj = s_sh // P      # 128-row blocks per (d, b)
    mul, add = mybir.AluOpType.mult, mybir.AluOpType.add
    es = ExitStack()

    singles = es.enter_context(tc.tile_pool(name="singles", bufs=1))
    ident = singles.tile([P, P], F32)
    masks.make_identity(nc, ident[:])
    ident_h = singles.tile([P, P], F16)
    masks.make_identity(nc, ident_h[:])
    # indicator stationaries: ind[:, d, m] = (m == d) / S — column-sums a
    # j-reduced fp16 plane into psum row d with one N=512 matmul per c-half.
    ind_h = singles.tile([P, D, D], F16)
    nc.vector.memset(ind_h[:], 0.0)
    for d in range(D):
        nc.vector.memset(ind_h[:, d, d:d + 1], 1.0 / (N_CORES * s_sh))
    w_sb = singles.tile([G, C], F32)
    nc.sync.dma_start(w_sb[:], w[:])
    # resident fp16 depths: xres[:, r] holds depth RES0+r; r=NRES-1 is d=D-1
    xres = singles.tile([P, NRES, B, nj, C], F16)
    gates_sb = singles.tile([P, B, nj, D], F32)
    meanT_sb = singles.tile([P, NKC * D], F32)
    wT_sb = singles.tile([P, NKC, G], F32)
    wT_h = singles.tile([P, NKC, G], F16)
    # per-b tiles: a shared tile would give phase-B readers a whole-tile RAW
    # hazard on the LAST writer (b=3's bounce / u-block), stalling b=0's gates
    keysT_sb = [singles.tile([G, D], F32, name=f"keysT{b}") for b in range(B)]
    uT_sb = [singles.tile([G, nj, P], F32, name=f"uT{b}") for b in range(B)]

    bstream = es.enter_context(tc.tile_pool(name="bstream", bufs=6))

    dram = es.enter_context(tc.tile_pool(name="dram", bufs=1, space="DRAM"))
    # tiny warm-up AllReduce: absorbs collective-comm setup under phase A
    ccw_in = dram.tile([1, 16], F32)
    ccw_out = dram.tile([1, 16], F32)
    cc_in = [dram.tile([G, D], F32, name=f"cc_in{b}") for b in range(B)]
    cc_out = [dram.tile([G, D], F32, name=f"cc_out{b}") for b in range(B)]
    warm_sb = singles.tile([1, 16], F32)
    nc.vector.memset(warm_sb[:], 0.0)
    nc.gpsimd.dma_start(ccw_in[:], warm_sb[:])
    nc.gpsimd.collective_compute(
        "AllReduce", add, replica_groups=[list(range(N_CORES))],
        ins=[ccw_in.opt()], outs=[ccw_out.opt()],
    )

    # ---------------- Phase A: batch-major streaming + per-b collective ----
    with tc.tile_pool(name="psumA", bufs=1, space="PSUM") as psA, \
         tc.tile_pool(name="psumT", bufs=1, space="PSUM") as psT, \
         tc.tile_pool(name="psumXA", bufs=3, space="PSUM") as psXA, \
         tc.tile_pool(name="psumU", bufs=2, space="PSUM") as psU, \
         tc.tile_pool(name="xtA", bufs=4) as xtA, \
         tc.tile_pool(name="stream", bufs=3) as stream, \
         tc.tile_pool(name="jtree", bufs=6) as jtree, \
         tc.tile_pool(name="sumsp", bufs=2) as sumsp, \
         tc.tile_pool(name="ksump", bufs=2) as ksump:

        # one-time W_u transpose: wT[c, g] chunks (fp32 + fp16 copies)
        for k in range(NKC):
            tr = psT.tile([P, NKC * D], F32, tag="fix")
            nc.tensor.transpose(
                tr[:, :G], w_sb[:, k * P:(k + 1) * P], ident[:G, :G])
            nc.vector.tensor_copy(wT_sb[:, k, :], tr[:, :G])
            nc.scalar.copy(wT_h[:, k, :], tr[:, :G])

        sums_ps = psA.tile([D, C], F32)

        def sum_plane(plane_h, d, first, last):
            # psum rows m != d get +0; one start/stop per 512-col bank per b
            for h in range(2):
                nc.tensor.matmul(
                    sums_ps[:, h * 512:(h + 1) * 512],
                    ind_h[:, d, :],
                    plane_h[:, h * 512:(h + 1) * 512],
                    start=first, stop=last,
                )

        def u_block(b, j):
            # uT[g, s-block] = sum_k (wT_k).T @ x7T_k on PE (reads resident
            # x7 fp16, so this can run any time after the d=D-1 cast)
            u_ps = psU.tile([G, P], F32, tag="u")
            for k in range(NKC):
                xt_ps = psXA.tile([P, P], F16, tag="xt_ps")
                nc.tensor.transpose(
                    xt_ps[:], xres[:, NRES - 1, b, j, k * P:(k + 1) * P],
                    ident_h[:])
                xt_sb = xtA.tile([P, P], F16, tag="xt_sb")
                # all bounce copies on DVE: they cost 822ns on ACT vs 261ns
                # here, and ACT is already carrying the resident casts
                nc.vector.tensor_copy(xt_sb[:], xt_ps[:])
                nc.tensor.matmul(
                    u_ps[:], wT_h[:, k, :], xt_sb[:],
                    start=(k == 0), stop=(k == NKC - 1))
            nc.vector.tensor_copy(uT_sb[b][:, j, :], u_ps[:])

        # depth order: d=7 first (fills resident x7 for u_blocks); the other
        # resident depths are spread mid-batch so their ACT cast bursts
        # don't pile up at the batch boundary
        dorder = [7, 0, 5, 1, 2, 6, 3, 4]
        assert sorted(dorder) == list(range(D))
        ublocks = [(b, j) for b in range(B) for j in range(nj)]
        ub_i = 0
        for b in range(B):
            for di, d in enumerate(dorder):
                slab = stream.tile([P, nj, C], F32, tag="slab")
                nc.sync.dma_start(
                    slab[:], x[d, b].rearrange("(j p) c -> p j c", p=P))
                t01 = jtree.tile([P, C], F16, tag="jt")
                t23 = jtree.tile([P, C], F16, tag="jt")
                tfin = jtree.tile([P, C], F16, tag="jt")
                if d >= RES0:
                    # resident: fp16 casts into xres on ACT, tree reads fp16
                    r = d - RES0
                    for j in range(nj):
                        nc.scalar.copy(xres[:, r, b, j, :], slab[:, j, :])
                    nc.vector.tensor_tensor(
                        t01[:], xres[:, r, b, 0, :], xres[:, r, b, 1, :], add)
                    nc.vector.tensor_tensor(
                        t23[:], xres[:, r, b, 2, :], xres[:, r, b, 3, :], add)
                else:
                    # transient: fp32 adds with fp16 outputs, no extra cast
                    nc.vector.tensor_tensor(
                        t01[:], slab[:, 0, :], slab[:, 1, :], add)
                    nc.vector.tensor_tensor(
                        t23[:], slab[:, 2, :], slab[:, 3, :], add)
                nc.vector.tensor_tensor(tfin[:], t01[:], t23[:], add)
                sum_plane(tfin[:], d, first=(di == 0), last=(di == D - 1))
                # interleave one uT block every other slab (its x7 input is
                # cast at di=0 of its own batch, so never dispatch ahead)
                if di % 2 == 1 and ub_i < len(ublocks) \
                        and ublocks[ub_i][0] <= b:
                    u_block(*ublocks[ub_i])
                    ub_i += 1

            # ---- per-b fixup: meanT transpose + partial keysT + AllReduce --
            sums_sb = sumsp.tile([D, C], F32, tag="sums")
            nc.vector.tensor_copy(sums_sb[:], sums_ps[:])
            mt_ps = psT.tile([P, NKC * D], F32, tag="fix")
            for k in range(NKC):
                nc.tensor.matmul(
                    mt_ps[:, k * D:(k + 1) * D],
                    sums_sb[:, k * P:(k + 1) * P], ident[:D, :D],
                    is_transpose=True, start=(k == 0), stop=(k == NKC - 1))
            nc.vector.tensor_copy(meanT_sb[:], mt_ps[:])
            keys_ps = psT.tile([P, NKC * D], F32, tag="fix")
            for k in range(NKC):
                nc.tensor.matmul(
                    keys_ps[:G, :D],
                    wT_sb[:, k, :],
                    meanT_sb[:, k * D:(k + 1) * D],
                    start=(k == 0), stop=(k == NKC - 1),
                )
            ksum_sb = ksump.tile([G, D], F32, tag="ksum")
            nc.vector.tensor_copy(ksum_sb[:], keys_ps[:G, :D])
            # gpsimd carries ONLY the collective chain: a collective_compute
            # parks its queue until the fabric completes, so no streaming
            # work may ride behind it
            nc.gpsimd.dma_start(cc_in[b][:], ksum_sb[:])
            nc.gpsimd.collective_compute(
                "AllReduce", add,
                replica_groups=[list(range(N_CORES))],
                ins=[cc_in[b].opt()], outs=[cc_out[b].opt()],
            )

    # ---------------- Phase B: gates + depth-weighted sum -------------------
    # The whole phase is emitted under a far-future wait_until fence: the
    # Tile scheduler's sim thinks collectives are fast and would otherwise
    # hoist collective-dependent phase-B ops (cc_out bounce, logits matmul)
    # into the middle of phase-A engine queues — parking ACT/PE on a 25-200us
    # fabric wait and freezing the stream pipeline behind them.
    es_b = ExitStack()
    es_b.enter_context(tc.tile_wait_until(1.0))
    with tc.tile_pool(name="psumL", bufs=2, space="PSUM") as psL, \
         tc.tile_pool(name="psumO", bufs=2, space="PSUM") as psO, \
         tc.tile_pool(name="acc16", bufs=3) as acc16p, \
         tc.tile_pool(name="accf", bufs=3) as accfp, \
         tc.tile_pool(name="bcast", bufs=8) as bcastp, \
         tc.tile_pool(name="small", bufs=8) as small:
        def emit_gates(b):
            # cc_out bounce on the scalar queue: collective b completed under
            # phase-A streaming (b=3's under phase B's b=2 streaming)
            nc.scalar.dma_start(keysT_sb[b][:], cc_out[b][:])
            for j in range(nj):
                lg_ps = psL.tile([P, D], F32, tag="lg")
                nc.tensor.matmul(lg_ps[:], uT_sb[b][:, j, :], keysT_sb[b][:])
                e_sb = small.tile([P, D], F32, tag="e")
                z_sb = small.tile([P, 1], F32, tag="z")
                rz_sb = small.tile([P, 1], F32, tag="rz")
                nc.scalar.activation(
                    e_sb[:], lg_ps[:], mybir.ActivationFunctionType.Exp,
                    accum_out=z_sb[:])
                nc.vector.reciprocal(rz_sb[:], z_sb[:])
                nc.scalar.mul(gates_sb[:, b, j, :], e_sb[:], rz_sb[:])

        # staggered fences freeze the intended order: with one fence time
        # the scheduler sim sees all 4 bounces "ready" at once and groups
        # them ahead of b=0's gates, parking the ACT queue on b=3's
        # collective. Gates are emitted ONE batch ahead of their combine
        # section so each batch boundary finds its gates precomputed.
        emit_gates(0)
        # y-writes are emitted one block late so the ACT queue never parks
        # next-block casts behind DVE's final combine of the previous block
        pending_write = None
        for b in range(B):
            tc.tile_set_cur_wait(1.0 + 0.5 * (b + 1))
            if b + 1 < B:
                emit_gates(b + 1)
            for j in range(nj):
                # streamed depths: ACT fuses the gate scale into the
                # fp32->fp16 downcast, then PE accumulates them with a plain
                # identity stationary into fp32 PSUM (DVE/ACT are
                # element-rate-bound at ~1.2us per [P, C] pass, so an 8-deep
                # elementwise chain can't keep up with DMA — PE + a short
                # 3-deep DVE chain for the resident depths can).
                out_ps = psO.tile([P, C], F32, tag="out")
                for d in range(RES0):
                    t = bstream.tile([P, C], F32, tag="bslab")
                    nc.sync.dma_start(t[:], x[d, b, j * P:(j + 1) * P, :])
                    th = bcastp.tile([P, C], F16, tag="bc")
                    nc.scalar.activation(
                        th[:], t[:], mybir.ActivationFunctionType.Copy,
                        scale=gates_sb[:, b, j, d:d + 1])
                    for h in range(2):
                        nc.tensor.matmul(
                            out_ps[:, h * 512:(h + 1) * 512],
                            ident_h[:], th[:, h * 512:(h + 1) * 512],
                            start=(d == 0), stop=(d == RES0 - 1))
                if pending_write is not None:
                    nc.scalar.dma_start(*pending_write)
                    pending_write = None
                # resident depths: short fp16 scalar_tensor_tensor chain
                acc16 = acc16p.tile([P, C], F16, tag="a16")
                nc.vector.tensor_scalar_mul(
                    acc16[:], xres[:, 0, b, j, :], gates_sb[:, b, j, RES0:RES0 + 1])
                for r in range(1, NRES):
                    d = RES0 + r
                    nc.vector.scalar_tensor_tensor(
                        out=acc16[:], in0=xres[:, r, b, j, :],
                        scalar=gates_sb[:, b, j, d:d + 1],
                        in1=acc16[:], op0=mul, op1=add)
                # final fp32 combine drains the PSUM group in the same op
                accf = accfp.tile([P, C], F32, tag="af")
                nc.vector.tensor_tensor(accf[:], out_ps[:], acc16[:], add)
                # y writes via the ACT HWDGE ring: sync keeps reading,
                # gpsimd keeps its collective queue clear
                pending_write = (y[b, j * P:(j + 1) * P, :], accf[:])
        nc.scalar.dma_start(*pending_write)

    es_b.close()
    es.close()


def build_nc(s_sh):
    nc = bacc.Bacc("TRN2", target_bir_lowering=False, debug=False,
                   num_devices=N_CORES)
    x_ap = nc.dram_tensor("x", [D, B, s_sh, C], F32, kind="ExternalInput").ap()
    w_ap = nc.dram_tensor("w", [G, C], F32, kind="ExternalInput").ap()
    y_ap = nc.dram_tensor("y", [B, s_sh, C], F32, kind="ExternalOutput").ap()
    with tile.TileContext(nc) as tc:
        build_body(tc, x_ap, w_ap, y_ap, s_sh)
    nc.compile()
    return nc


_NC_CACHE = {}


def _get_nc(s_sh):
    if s_sh not in _NC_CACHE:
        _NC_CACHE[s_sh] = build_nc(s_sh)
    return _NC_CACHE[s_sh]


def run(cached_states, W_u, trace=False, trace_cores=None):
    s_sh = S // N_CORES
    nc = _get_nc(s_sh)
    xs = np.asarray(cached_states, dtype=np.float32)
    ws = np.ascontiguousarray(np.asarray(W_u, dtype=np.float32))
    in_maps = []
    for i in range(N_CORES):
        sh = np.ascontiguousarray(xs[:, :, i * s_sh:(i + 1) * s_sh, :])
        in_maps.append({"x": sh, "w": ws})
    res = bass_utils.run_bass_kernel_spmd(
        nc, in_maps, core_ids=list(range(N_CORES)), trace=trace,
        trace_cores=trace_cores)
    out = np.empty((B, S, C), np.float32)
    for i in range(N_CORES):
        out[:, i * s_sh:(i + 1) * s_sh, :] = res.results[i]["y"]
    return out, res


def kernel(cached_states, W_u):
    out, _ = run(cached_states, W_u)
    return out
